# revision 2
# baseline (speedup 1.0000x reference)
"""CIN (Compressed Interaction Network) kernel for Trainium2, 8-core data parallel.

Math (per batch row b, embedding dim d — R = B*D independent rows):
  layer k: cur_k[m, (b,d)] = sum_{f,g} W_k[f*G+g, m] * x0[f,(b,d)] * x_{k}[g,(b,d)]
  output  = concat_k( sum_d cur_k )    -> [B, 384]

Device strategy (per core, batch-sharded B/8 = 256 -> R = 4096 rows):
  * Everything lives feature-on-partitions: cur_k^T [128, R] etc.
  * Layer 0 uses the x (x) x symmetry: W0 is host-symmetrized to the upper
    triangle (k: 1521 -> 780, padded 896) and the two z factors are
    host-gathered index patterns of x^T (pure indexing, no arithmetic).
  * Layer 1 uses a PAIR-SPLIT of the (f, g) contraction index:
      pair (f, g) -> partition p = (f_lo in 8, g_lo in 16),
                     k-tile  t = (f_hi in 5, g_hi in 8)   (f padded 39->40)
    so the z build z[p, t, l] = xrep[p, f_hi, l] * cur0rep[p, g_hi, l]
    needs x replicated only 16x (host layout) and cur0 replicated only 8x
    (on-device DRAM bounce + broadcast-read DMA), instead of the naive
    x-broadcast to all 128 partitions (128x replication, 41 MB of DMA).
    W1 is host-permuted to match the (t, p) row order.
  * Layer 2's output only needs sum_d, so it uses the Gram trick: per-batch
    P12[b,f,g] = sum_d x[b,f,d] cur1[b,g,d] built on the PE with a
    block-diagonal x operand, then out2 = W2^T vec(P12) (stage B).
  * z-multiplies are split DVE / GpSimd to balance the two engines; DMA
    issues ride the cheap GpSimd sequencer where no upstream wait exists.
  * All DRAM operands are laid out chunk-major on the host so every DMA has
    large contiguous per-partition runs (big packets -> full DMA bandwidth).
"""

import sys
import types

sys.path.insert(0, "/opt/trn_rl_repo")

# The image's antenv package lacks axon_hooks; bass_utils imports it if
# BASS_TRACE is set in the environment. Seed a benign stub so that path
# degrades to "no tracing" instead of ModuleNotFoundError.
if "antenv.axon_hooks" not in sys.modules:
    _ah = types.ModuleType("antenv.axon_hooks")
    _ah.get_axon_ntff_profile_hook = lambda: None
    _ah.set_axon_ntff_profile_hook = lambda h: None
    sys.modules["antenv.axon_hooks"] = _ah

import numpy as np
import ml_dtypes

import concourse.bass as bass
import concourse.mybir as mybir
from concourse import bacc
from concourse.tile import TileContext
from concourse.bass_utils import run_bass_kernel_spmd

BF16 = ml_dtypes.bfloat16

B, F0, D = 2048, 39, 16
M = 128                      # layer width (all three layers)
NCORES = 8
BPC = B // NCORES            # batch per core = 256
R = BPC * D                  # rows per core = 4096
K0 = (F0 * (F0 + 1)) // 2    # 780 (triangular)
K0P = 896                    # padded to 7 k-tiles
NKT0 = K0P // 128            # 7

# layer-1 pair split: p = (f_lo, g_lo), t = (f_hi, g_hi)
PF = 8                       # f_lo values per partition group
PG = 16                      # g_lo values per partition group
TF = 5                       # f_hi tiles (f padded 39 -> 40)
TG = 8                       # g_hi tiles (g = 128)
NKT1 = TF * TG               # 40 k-tiles

L = 512                      # bd-chunk (32 b x 16 d)
NCHUNK = R // L              # 8
BPCH = L // D                # 32 batches per chunk
NTILE = L // 128             # 4 bd-tiles of 128 rows per chunk

DT = mybir.dt.bfloat16
DTF = mybir.dt.float32

_CACHE = {}


def _build_program():
    nc = bacc.Bacc("TRN2", target_bir_lowering=False, debug=False,
                   num_devices=NCORES)

    # chunk-major layouts so per-partition DMA runs are contiguous
    zin0 = nc.declare_dram_parameter("zin0", [NCHUNK, 128, NKT0, L], DT,
                                     isOutput=False)
    zin1 = nc.declare_dram_parameter("zin1", [NCHUNK, 128, NKT0, L], DT,
                                     isOutput=False)
    xrep = nc.declare_dram_parameter("xrep", [NCHUNK, 128, TF, L], DT,
                                     isOutput=False)
    w0 = nc.declare_dram_parameter("w0", [K0P, M], DT, isOutput=False)
    w1p = nc.declare_dram_parameter("w1p", [NKT1 * 128, M], DT, isOutput=False)
    w2 = nc.declare_dram_parameter("w2", [F0 * M, M], DT, isOutput=False)
    ident = nc.declare_dram_parameter("ident", [128, 128], DTF, isOutput=False)
    identb = nc.declare_dram_parameter("identb", [128, 128], DT, isOutput=False)
    xbd = nc.declare_dram_parameter("xbd", [NCHUNK, 128, NTILE, 9 * F0], DT,
                                    isOutput=False)
    out = nc.declare_dram_parameter("out", [BPC, 3 * M], DTF, isOutput=True)

    # on-device bounce buffer for the cur0 8x partition replication
    c0scr = nc.dram_tensor("c0scr", (NCHUNK, 128, L), DT, kind="Internal")

    with TileContext(nc) as tc:
        with (
            tc.tile_pool(name="wpool", bufs=1) as wpool,
            tc.tile_pool(name="zin", bufs=2) as zinpool,
            tc.tile_pool(name="rep", bufs=2) as reppool,
            tc.tile_pool(name="zt", bufs=3) as zpool,
            tc.tile_pool(name="cur", bufs=3) as curpool,
            tc.tile_pool(name="outp", bufs=1) as outpool,
            tc.tile_pool(name="psum", bufs=3, space="PSUM") as pspool,
            tc.tile_pool(name="pst", bufs=1, space="PSUM") as pstpool,
            tc.tile_pool(name="pstb", bufs=2, space="PSUM") as pstbpool,
            tc.tile_pool(name="psa", bufs=1, space="PSUM") as psapool,
            tc.tile_pool(name="pso2", bufs=1, space="PSUM") as pso2pool,
        ):
            def issue_chunk_dmas(c, pieces=1):
                """Prefetch one chunk's L0 factors, L1 x-factor and L2
                block-diag x. No upstream deps -> ride the (cheap) gpsimd
                DMA-issue path. pieces>1 splits the startup transfer."""
                zin0t = zinpool.tile([128, NKT0, L], DT, tag="zin0",
                                     name=f"zin0_{c}")
                zin1t = zinpool.tile([128, NKT0, L], DT, tag="zin1",
                                     name=f"zin1_{c}")
                if pieces > 1:
                    nc.gpsimd.dma_start(out=zin0t[:, :4, :], in_=zin0[c, :, :4, :])
                    nc.gpsimd.dma_start(out=zin1t[:, :4, :], in_=zin1[c, :, :4, :])
                    nc.gpsimd.dma_start(out=zin0t[:, 4:, :], in_=zin0[c, :, 4:, :])
                    nc.gpsimd.dma_start(out=zin1t[:, 4:, :], in_=zin1[c, :, 4:, :])
                else:
                    nc.gpsimd.dma_start(out=zin0t[:], in_=zin0[c])
                    nc.gpsimd.dma_start(out=zin1t[:], in_=zin1[c])
                xrt = zinpool.tile([128, TF, L], DT, tag="xrep",
                                   name=f"xrep_{c}")
                nc.gpsimd.dma_start(out=xrt[:], in_=xrep[c])
                xbdt = zinpool.tile([128, NTILE, 9 * F0], DT, tag="xbd",
                                    name=f"xbd_{c}")
                nc.gpsimd.dma_start(out=xbdt[:], in_=xbd[c])
                return zin0t, zin1t, xrt, xbdt

            G0 = [4, 3]          # layer-0 k-tile TT groups (sum NKT0)

            # ---- small weights first (L0's matmuls need w0 immediately),
            # then the chunk-0 prefetch; w1p/w2 follow on the sync queue.
            w0s = wpool.tile([128, NKT0, M], DT, tag="w0")
            nc.sync.dma_start(out=w0s[:], in_=w0.rearrange("(t p) m -> p t m", p=128))
            ids = wpool.tile([128, 128], DTF, tag="ident")
            nc.scalar.dma_start(out=ids[:], in_=ident[:])
            idb = wpool.tile([128, 128], DT, tag="identb")
            nc.scalar.dma_start(out=idb[:], in_=identb[:])
            pref = issue_chunk_dmas(0, pieces=2)
            w1s = wpool.tile([128, NKT1, M], DT, tag="w1")
            nc.sync.dma_start(out=w1s[:], in_=w1p.rearrange("(t p) m -> p t m", p=128))
            w2s = wpool.tile([128, F0, M], DT, tag="w2")
            nc.scalar.dma_start(out=w2s[:], in_=w2.rearrange("(t p) m -> p t m", p=128))

            # per-layer output accumulators [128 m, BPC] fp32
            outacc = [
                outpool.tile([128, BPC], DTF, tag=f"oacc{k}", name=f"oacc{k}")
                for k in range(3)
            ]

            def do_l0(c, zin0t, zin1t):
                """Layer 0 for chunk c: z0 = zin0*zin1 (DVE + gpsimd),
                matmuls -> ps0; reduce -> out0; copy + replicate cur0."""
                ps0 = pspool.tile([128, L], DTF, tag="ps", name=f"ps0_{c}")
                kt = 0
                for gi, gsz in enumerate(G0):
                    z0t = zpool.tile([128, gsz, L], DT, tag="z",
                                     name=f"z0_{c}_{kt}")
                    eng = nc.vector if gi == 0 else nc.gpsimd
                    eng.tensor_mul(
                        z0t[:],
                        zin0t[:, kt : kt + gsz, :],
                        zin1t[:, kt : kt + gsz, :],
                    )
                    for j in range(gsz):
                        nc.tensor.matmul(
                            ps0[:],
                            w0s[:, kt + j, :],
                            z0t[:, j, :],
                            start=(kt + j == 0),
                            stop=(kt + j == NKT0 - 1),
                        )
                    kt += gsz
                nc.vector.tensor_reduce(
                    outacc[0][:, c * BPCH : (c + 1) * BPCH],
                    ps0[:].rearrange("p (b d) -> p b d", d=D),
                    axis=mybir.AxisListType.X,
                    op=mybir.AluOpType.add,
                )
                cur0 = curpool.tile([128, L], DT, tag="cur0", name=f"cur0_{c}")
                nc.scalar.copy(cur0[:], ps0[:])
                # replicate cur0 8x across partition f_lo-groups via a DRAM
                # bounce: SBUF -> DRAM, then 8 broadcast reads back.
                nc.sync.dma_start(out=c0scr[c], in_=cur0[:])
                rept = reppool.tile([128, TG, L], DT, tag="c0rep",
                                    name=f"c0rep_{c}")
                sview = c0scr[c].rearrange("(gh gl) l -> gl gh l", gl=PG)
                for f_lo in range(PF):
                    eng = nc.sync if f_lo % 2 == 0 else nc.scalar
                    eng.dma_start(
                        out=rept[f_lo * PG : (f_lo + 1) * PG, :, :], in_=sview
                    )
                return rept

            def do_l1(c, rept, xrt):
                """Layer 1 for chunk c via the pair split."""
                ps = pspool.tile([128, L], DTF, tag="ps", name=f"ps1_{c}")
                for fh in range(TF):
                    zt = zpool.tile([128, TG, L], DT, tag="z",
                                    name=f"z1_{c}_{fh}")
                    xb = xrt[:, fh, :].unsqueeze(1)
                    if fh >= 3:
                        # split the mul DVE/gpsimd to balance engines
                        nc.vector.tensor_mul(
                            zt[:, 0:6, :], rept[:, 0:6, :],
                            xb.to_broadcast((128, 6, L)),
                        )
                        nc.gpsimd.tensor_mul(
                            zt[:, 6:8, :], rept[:, 6:8, :],
                            xb.to_broadcast((128, 2, L)),
                        )
                    else:
                        nc.vector.tensor_mul(
                            zt[:], rept[:], xb.to_broadcast((128, TG, L)),
                        )
                    for gh in range(TG):
                        t = fh * TG + gh
                        nc.tensor.matmul(
                            ps[:],
                            w1s[:, t, :],
                            zt[:, gh, :],
                            start=(t == 0),
                            stop=(t == NKT1 - 1),
                        )
                nc.vector.tensor_reduce(
                    outacc[1][:, c * BPCH : (c + 1) * BPCH],
                    ps[:].rearrange("p (b d) -> p b d", d=D),
                    axis=mybir.AxisListType.X,
                    op=mybir.AluOpType.add,
                )
                cur1 = curpool.tile([128, L], DT, tag="cur1", name=f"cur1_{c}")
                nc.scalar.copy(cur1[:], ps[:])
                return cur1

            # layer 2 via the d-contraction (Gram) trick: the final output
            # only needs sum_d cur2, and
            #   out2[b, m] = sum_{f,g} W2[fg, m] * P12[b, f, g],
            #   P12[b, f, g] = sum_d x[b, f, d] * cur1[b, g, d].
            # P12 is built on the PE with a block-diagonal x operand (bd rows
            # on partitions, contraction over the 16 d's of each batch).
            p12 = outpool.tile([128, NCHUNK * NTILE * 9 * F0], DT, tag="p12")

            def do_l2p(c, cur1, xbdt):
                for t in range(NTILE):
                    pstc = pstbpool.tile([128, 128], DT, tag="pstb",
                                        name=f"pstc_{c}_{t}")
                    nc.tensor.transpose(
                        pstc[:], cur1[:, t * 128 : (t + 1) * 128], idb[:]
                    )
                    c1bd = curpool.tile([128, 128], DT, tag="c1bd",
                                        name=f"c1bd_{c}_{t}")
                    nc.scalar.copy(c1bd[:], pstc[:])
                    psa = psapool.tile([128, 9 * F0], DTF, tag="psa",
                                       name=f"psa_{c}_{t}")
                    nc.tensor.matmul(
                        psa[:], c1bd[:], xbdt[:, t, :],
                        start=True, stop=True,
                    )
                    off = (c * NTILE + t) * 9 * F0
                    nc.scalar.copy(p12[:, off : off + 9 * F0], psa[:])

            def flush_out(h):
                for k in range(3):
                    pst = pstpool.tile([128, 128], DTF, tag="pst",
                                       name=f"pst_{k}_{h}")
                    nc.tensor.transpose(
                        pst[:], outacc[k][:, h * 128 : (h + 1) * 128], ids[:]
                    )
                    ot = curpool.tile([128, 128], DTF, tag="otile",
                                      name=f"ot_{k}_{h}")
                    nc.scalar.copy(ot[:], pst[:])
                    nc.scalar.dma_start(
                        out=out[h * 128 : (h + 1) * 128, k * M : (k + 1) * M],
                        in_=ot[:],
                    )

            # stage B of the L2 Gram trick, over a half-range of chunks:
            # out2^T[m, b-half] = sum_f w2_f^T @ p12[:, (c, t, b, f)]
            pso2 = pso2pool.tile([128, BPC], DTF, tag="pso2")
            p12v = p12[:].rearrange("p (ct s f) -> p ct s f", s=9, f=F0)

            def stage_b(c0, c1):
                nt0, nt1 = c0 * NTILE, c1 * NTILE
                b0, b1 = c0 * BPCH, c1 * BPCH
                for f in range(F0):
                    nc.tensor.matmul(
                        pso2[:, b0:b1], w2s[:, f, :],
                        p12v[:, nt0:nt1, 0:8, f],
                        start=(f == 0), stop=(f == F0 - 1),
                    )
                nc.scalar.copy(outacc[2][:, b0:b1], pso2[:, b0:b1])

            # software pipeline: chunk c+1's L0 runs between chunk c's L1 and
            # the L2 P-build so the PE/DVE never drain at chunk boundaries,
            # and cur0's replication DMA hides under L2P + prefetches.
            zin0_c, zin1_c, xrt_c, xbdt_c = pref
            rept_c = do_l0(0, zin0_c, zin1_c)
            for c in range(NCHUNK):
                if 1 <= c < NCHUNK - 1:
                    zin0_n, zin1_n, xrt_n, xbdt_n = issue_chunk_dmas(c + 1)
                cur1_c = do_l1(c, rept_c, xrt_c)
                if c == 0:
                    zin0_n, zin1_n, xrt_n, xbdt_n = issue_chunk_dmas(1)
                if c + 1 < NCHUNK:
                    rept_n = do_l0(c + 1, zin0_n, zin1_n)
                do_l2p(c, cur1_c, xbdt_c)
                if c == 3:
                    stage_b(0, 4)
                if c == 5:
                    stage_b(4, 6)
                if c + 1 < NCHUNK:
                    rept_c, xrt_c, xbdt_c = rept_n, xrt_n, xbdt_n

            # stage B last quarter; earlier parts were emitted inside the loop.
            stage_b(6, 8)
            flush_out(0)
            flush_out(1)


    nc.compile()
    return nc


def _host_prep(inputs, f0, f1, f2):
    """Per-core input maps. Pure layout/cast/index-gather, no FLOP offload
    (except the W0 symmetrization, which is weight preprocessing)."""
    x = np.asarray(inputs)

    # symmetrized triangular W0: rows (f, g) f<=g
    f0n = np.asarray(f0).reshape(F0, F0, M)
    fi, gi = np.triu_indices(F0)
    w0t = f0n[fi, gi] + np.where((fi != gi)[:, None], f0n[gi, fi], 0.0)
    w0 = np.zeros((K0P, M), dtype=BF16)
    w0[:K0] = w0t.astype(BF16)

    # layer-1 weights permuted to the pair-split (t, p) row order
    w1 = np.asarray(f1).astype(np.float32)
    tt = np.repeat(np.arange(NKT1), 128)
    pp = np.tile(np.arange(128), NKT1)
    fidx1 = (tt // TG) * PF + pp // PG
    gidx1 = (tt % TG) * PG + pp % PG
    valid1 = fidx1 < F0
    rows = np.clip(fidx1, 0, F0 - 1) * M + gidx1
    w1perm = np.where(valid1[:, None], w1[rows], 0.0).astype(BF16)

    w2 = np.asarray(f2).astype(BF16)
    ident = np.eye(128, dtype=np.float32)
    identb = np.eye(128, dtype=BF16)

    # layer-0 z-factor gather indices (triangular, k-row = tile*128 + p)
    pidx = np.arange(K0P)
    fidx = np.zeros(K0P, np.int64)
    gidx = np.zeros(K0P, np.int64)
    fidx[:K0], gidx[:K0] = fi, gi
    valid = (pidx < K0).astype(BF16)[:, None]

    # layer-1 xrep row map: xrep[p, f_hi] = x row f_hi*PF + p//PG (<F0)
    prow = np.arange(128)
    fmap = np.arange(TF)[None, :] * PF + (prow // PG)[:, None]   # [128, TF]
    vmap = (fmap < F0)[..., None]

    maps = []
    for c in range(NCORES):
        xs = x[c * BPC : (c + 1) * BPC]                    # [256, 39, 16]
        xTf = np.ascontiguousarray(
            xs.transpose(1, 0, 2).reshape(F0, R)
        ).astype(BF16)                                     # [39, R]
        # layer-0 factors [K0P, R] -> chunk-major [NCHUNK, 128, NKT0, L]
        z0a = (xTf[gidx] * valid).reshape(NKT0, 128, NCHUNK, L)
        z0b = (xTf[fidx] * valid).reshape(NKT0, 128, NCHUNK, L)
        zin0c = np.ascontiguousarray(z0a.transpose(2, 1, 0, 3))
        zin1c = np.ascontiguousarray(z0b.transpose(2, 1, 0, 3))
        # layer-1 x-factor, 16x partition-replicated: [NCHUNK, 128, TF, L]
        xg = np.where(vmap, xTf[np.clip(fmap, 0, F0 - 1)], BF16(0))
        xrepc = np.ascontiguousarray(
            xg.reshape(128, TF, NCHUNK, L).transpose(2, 0, 1, 3)
        )
        xbd_full = xs.transpose(0, 2, 1).reshape(R, F0).astype(BF16)
        xbdt = xbd_full.reshape(NCHUNK, NTILE, 128, F0).transpose(0, 2, 1, 3)
        xbdh = np.zeros((NCHUNK, 128, NTILE, 9 * F0), dtype=BF16)
        p = np.arange(128)
        for s in range(8):
            rows_s = p[p // 16 == s]
            xbdh[:, rows_s, :, s * F0 : (s + 1) * F0] = xbdt[:, rows_s, :, :]
        maps.append(
            dict(zin0=zin0c, zin1=zin1c, w0=w0, w1p=w1perm, w2=w2,
                 ident=ident, identb=identb, xbd=xbdh, xrep=xrepc)
        )
    return maps


def kernel(**inputs) -> np.ndarray:
    if "nc" not in _CACHE:
        _CACHE["nc"] = _build_program()
    nc = _CACHE["nc"]
    maps = _host_prep(inputs["inputs"], inputs["f0"], inputs["f1"], inputs["f2"])
    res = run_bass_kernel_spmd(nc, maps, list(range(NCORES)))
    return np.concatenate([res.results[c]["out"] for c in range(NCORES)], axis=0)


if __name__ == "__main__":
    rng = np.random.default_rng(0)
    ins = {
        "inputs": rng.standard_normal((B, F0, D), dtype=np.float32),
        "f0": (rng.standard_normal((F0 * F0, M)) * 0.05).astype(np.float32),
        "f1": (rng.standard_normal((F0 * M, M)) * 0.05).astype(np.float32),
        "f2": (rng.standard_normal((F0 * M, M)) * 0.05).astype(np.float32),
    }
    out = kernel(**ins)
    print("out", out.shape, out.dtype)


# revision 3
# speedup vs baseline: 1.1706x; 1.1706x over previous
"""CIN (Compressed Interaction Network) kernel for Trainium2, 8-core data parallel.

Math (per batch row b, embedding dim d — R = B*D independent rows):
  layer k: cur_k[m, (b,d)] = sum_{f,g} W_k[f*G+g, m] * x0[f,(b,d)] * x_{k}[g,(b,d)]
  output  = concat_k( sum_d cur_k )    -> [B, 384]

Device strategy (per core, batch-sharded B/8 = 256 -> R = 4096 rows):
  * Everything lives feature-on-partitions: cur_k^T [128, R] etc.
  * Layer 0 uses the x (x) x symmetry: W0 is host-symmetrized to the upper
    triangle (k: 1521 -> 780, padded 896) and the two z factors are
    host-gathered index patterns of x^T (pure indexing, no arithmetic).
  * Layer 1 uses a PAIR-SPLIT of the (f, g) contraction index:
      pair (f, g) -> partition p = (f_lo in 4, g_lo in 32),
                     k-tile  t = (f_hi in 10, g_hi in 4)   (f padded 39->40)
    so the z build z[p, t, l] = xrep[p, f_hi, l] * cur0rep[p, g_hi, l]
    needs x replicated only 32x (host layout) and cur0 replicated only 4x
    (on-device DRAM bounce + 4 broadcast-read DMAs), instead of the naive
    x-broadcast to all 128 partitions (128x replication, 41 MB of DMA).
    W1 is host-permuted to match the (t, p) row order.
  * Layer 2's output only needs sum_d, so it uses the Gram trick: per-batch
    P12[b,f,g] = sum_d x[b,f,d] cur1[b,g,d] built on the PE with a
    block-diagonal x operand, then out2 = W2^T vec(P12) (stage B).
  * z-multiplies are split DVE / GpSimd to balance the two engines.
  * Queue discipline: ALL per-chunk dma_starts ride the sync (SP) sequencer
    (it has no compute duties, so its in-order waits block nothing); the
    Act sequencer only issues engine copies. GpSimd issues no DMAs (SWDGE
    prep costs ~1us of Pool engine time each).
  * Software pipeline: inputs prefetched 2 chunks ahead; chunk c+1's L0 is
    interleaved into the middle of chunk c's L1 matmul stream so the cur0
    bounce/replication DMA latency hides under the remaining L1 groups.
"""

import sys
import types

sys.path.insert(0, "/opt/trn_rl_repo")

# The image's antenv package lacks axon_hooks; bass_utils imports it if
# BASS_TRACE is set in the environment. Seed a benign stub so that path
# degrades to "no tracing" instead of ModuleNotFoundError.
if "antenv.axon_hooks" not in sys.modules:
    _ah = types.ModuleType("antenv.axon_hooks")
    _ah.get_axon_ntff_profile_hook = lambda: None
    _ah.set_axon_ntff_profile_hook = lambda h: None
    sys.modules["antenv.axon_hooks"] = _ah

import numpy as np
import ml_dtypes

import concourse.bass as bass
import concourse.mybir as mybir
from concourse import bacc
from concourse.tile import TileContext
from concourse.bass_utils import run_bass_kernel_spmd

BF16 = ml_dtypes.bfloat16

B, F0, D = 2048, 39, 16
M = 128                      # layer width (all three layers)
NCORES = 8
BPC = B // NCORES            # batch per core = 256
R = BPC * D                  # rows per core = 4096
K0 = (F0 * (F0 + 1)) // 2    # 780 (triangular)
K0P = 896                    # padded to 7 k-tiles
NKT0 = K0P // 128            # 7

# layer-1 pair split: p = (f_lo, g_lo), t = (f_hi, g_hi)
PF = 4                       # f_lo values per partition group
PG = 32                      # g_lo values per partition group
TF = 10                      # f_hi tiles (f padded 39 -> 40)
TG = 4                       # g_hi tiles (g = 128)
NKT1 = TF * TG               # 40 k-tiles

L = 512                      # bd-chunk (32 b x 16 d)
NCHUNK = R // L              # 8
BPCH = L // D                # 32 batches per chunk
NTILE = L // 128             # 4 bd-tiles of 128 rows per chunk

DT = mybir.dt.bfloat16
DTF = mybir.dt.float32

_CACHE = {}


def _build_program():
    nc = bacc.Bacc("TRN2", target_bir_lowering=False, debug=False,
                   num_devices=NCORES)

    # chunk-major layouts so per-partition DMA runs are contiguous
    zin0 = nc.declare_dram_parameter("zin0", [NCHUNK, 128, NKT0, L], DT,
                                     isOutput=False)
    zin1 = nc.declare_dram_parameter("zin1", [NCHUNK, 128, NKT0, L], DT,
                                     isOutput=False)
    xrep = nc.declare_dram_parameter("xrep", [NCHUNK, 128, TF, L], DT,
                                     isOutput=False)
    w0 = nc.declare_dram_parameter("w0", [K0P, M], DT, isOutput=False)
    w1p = nc.declare_dram_parameter("w1p", [NKT1 * 128, M], DT, isOutput=False)
    w2 = nc.declare_dram_parameter("w2", [F0 * M, M], DT, isOutput=False)
    ident = nc.declare_dram_parameter("ident", [128, 128], DTF, isOutput=False)
    identb = nc.declare_dram_parameter("identb", [128, 128], DT, isOutput=False)
    xbd = nc.declare_dram_parameter("xbd", [NCHUNK, 128, NTILE, 9 * F0], DT,
                                    isOutput=False)
    out = nc.declare_dram_parameter("out", [BPC, 3 * M], DTF, isOutput=True)

    # on-device bounce buffer for the cur0 4x partition replication
    c0scr = nc.dram_tensor("c0scr", (NCHUNK, 128, L), DT, kind="Internal")

    with TileContext(nc) as tc:
        with (
            tc.tile_pool(name="wpool", bufs=1) as wpool,
            tc.tile_pool(name="zin", bufs=3) as zinpool,
            tc.tile_pool(name="rep", bufs=2) as reppool,
            tc.tile_pool(name="zt", bufs=3) as zpool,
            tc.tile_pool(name="cur", bufs=3) as curpool,
            tc.tile_pool(name="outp", bufs=1) as outpool,
            tc.tile_pool(name="psum", bufs=3, space="PSUM") as pspool,
            tc.tile_pool(name="pst", bufs=1, space="PSUM") as pstpool,
            tc.tile_pool(name="pstb", bufs=2, space="PSUM") as pstbpool,
            tc.tile_pool(name="psa", bufs=1, space="PSUM") as psapool,
            tc.tile_pool(name="pso2", bufs=1, space="PSUM") as pso2pool,
        ):
            def issue_chunk_dmas(c, pieces=1):
                """Prefetch one chunk's L0 factors, L1 x-factor and L2
                block-diag x. No upstream waits -> safe anywhere on sync."""
                zin0t = zinpool.tile([128, NKT0, L], DT, tag="zin0",
                                     name=f"zin0_{c}")
                zin1t = zinpool.tile([128, NKT0, L], DT, tag="zin1",
                                     name=f"zin1_{c}")
                if pieces > 1:
                    nc.sync.dma_start(out=zin0t[:, :4, :], in_=zin0[c, :, :4, :])
                    nc.sync.dma_start(out=zin1t[:, :4, :], in_=zin1[c, :, :4, :])
                    nc.sync.dma_start(out=zin0t[:, 4:, :], in_=zin0[c, :, 4:, :])
                    nc.sync.dma_start(out=zin1t[:, 4:, :], in_=zin1[c, :, 4:, :])
                else:
                    nc.sync.dma_start(out=zin0t[:], in_=zin0[c])
                    nc.sync.dma_start(out=zin1t[:], in_=zin1[c])
                xrt = zinpool.tile([128, TF, L], DT, tag="xrep",
                                   name=f"xrep_{c}")
                nc.sync.dma_start(out=xrt[:], in_=xrep[c])
                xbdt = zinpool.tile([128, NTILE, 9 * F0], DT, tag="xbd",
                                    name=f"xbd_{c}")
                nc.sync.dma_start(out=xbdt[:], in_=xbd[c])
                return zin0t, zin1t, xrt, xbdt

            G0 = [4, 3]          # layer-0 k-tile TT groups (sum NKT0)

            # ---- small weights first (L0's matmuls need w0 immediately),
            # then the chunk-0 prefetch; w1p/w2 follow on the sync queue.
            w0s = wpool.tile([128, NKT0, M], DT, tag="w0")
            nc.sync.dma_start(out=w0s[:], in_=w0.rearrange("(t p) m -> p t m", p=128))
            ids = wpool.tile([128, 128], DTF, tag="ident")
            nc.scalar.dma_start(out=ids[:], in_=ident[:])
            idb = wpool.tile([128, 128], DT, tag="identb")
            nc.scalar.dma_start(out=idb[:], in_=identb[:])
            pref = issue_chunk_dmas(0, pieces=2)
            w1s = wpool.tile([128, NKT1, M], DT, tag="w1")
            nc.sync.dma_start(out=w1s[:], in_=w1p.rearrange("(t p) m -> p t m", p=128))
            w2s = wpool.tile([128, F0, M], DT, tag="w2")
            nc.scalar.dma_start(out=w2s[:], in_=w2.rearrange("(t p) m -> p t m", p=128))

            # per-layer output accumulators [128 m, BPC] fp32
            outacc = [
                outpool.tile([128, BPC], DTF, tag=f"oacc{k}", name=f"oacc{k}")
                for k in range(3)
            ]

            def do_l0A(c, zin0t, zin1t):
                """Layer 0 front half: z0 = zin0*zin1 (DVE + gpsimd),
                7 PSUM-accumulated matmuls."""
                ps0 = pspool.tile([128, L], DTF, tag="ps", name=f"ps0_{c}")
                kt = 0
                for gi, gsz in enumerate(G0):
                    z0t = zpool.tile([128, gsz, L], DT, tag="z",
                                     name=f"z0_{c}_{kt}")
                    eng = nc.vector if gi == 0 else nc.gpsimd
                    eng.tensor_mul(
                        z0t[:],
                        zin0t[:, kt : kt + gsz, :],
                        zin1t[:, kt : kt + gsz, :],
                    )
                    for j in range(gsz):
                        nc.tensor.matmul(
                            ps0[:],
                            w0s[:, kt + j, :],
                            z0t[:, j, :],
                            start=(kt + j == 0),
                            stop=(kt + j == NKT0 - 1),
                        )
                    kt += gsz
                return ps0

            def do_l0B(c, ps0):
                """Layer 0 back half: out0 reduce, cur0 copy, and the 4x
                partition replication via a DRAM bounce (all DMAs on sync)."""
                nc.vector.tensor_reduce(
                    outacc[0][:, c * BPCH : (c + 1) * BPCH],
                    ps0[:].rearrange("p (b d) -> p b d", d=D),
                    axis=mybir.AxisListType.X,
                    op=mybir.AluOpType.add,
                )
                cur0 = curpool.tile([128, L], DT, tag="cur0", name=f"cur0_{c}")
                nc.scalar.copy(cur0[:], ps0[:])
                nc.sync.dma_start(out=c0scr[c], in_=cur0[:])
                rept = reppool.tile([128, TG, L], DT, tag="c0rep",
                                    name=f"c0rep_{c}")
                sview = c0scr[c].rearrange("(gh gl) l -> gl gh l", gl=PG)
                for f_lo in range(PF):
                    nc.sync.dma_start(
                        out=rept[f_lo * PG : (f_lo + 1) * PG, :, :], in_=sview
                    )
                return rept

            def l1_group(c, ps, rept, xrt, fh):
                """One f_hi group of layer 1: z build + TG matmuls."""
                zt = zpool.tile([128, TG, L], DT, tag="z", name=f"z1_{c}_{fh}")
                xb = xrt[:, fh, :].unsqueeze(1)
                if fh >= TF - 2:
                    # split the mul DVE/gpsimd to balance engines
                    nc.vector.tensor_mul(
                        zt[:, 0:2, :], rept[:, 0:2, :],
                        xb.to_broadcast((128, 2, L)),
                    )
                    nc.gpsimd.tensor_mul(
                        zt[:, 2:4, :], rept[:, 2:4, :],
                        xb.to_broadcast((128, 2, L)),
                    )
                else:
                    nc.vector.tensor_mul(
                        zt[:], rept[:], xb.to_broadcast((128, TG, L)),
                    )
                for gh in range(TG):
                    t = fh * TG + gh
                    nc.tensor.matmul(
                        ps[:],
                        w1s[:, t, :],
                        zt[:, gh, :],
                        start=(t == 0),
                        stop=(t == NKT1 - 1),
                    )

            def do_l1B(c, ps):
                """Layer 1 tail: out1 reduce + cur1 copy."""
                nc.vector.tensor_reduce(
                    outacc[1][:, c * BPCH : (c + 1) * BPCH],
                    ps[:].rearrange("p (b d) -> p b d", d=D),
                    axis=mybir.AxisListType.X,
                    op=mybir.AluOpType.add,
                )
                cur1 = curpool.tile([128, L], DT, tag="cur1", name=f"cur1_{c}")
                nc.scalar.copy(cur1[:], ps[:])
                return cur1

            # layer 2 via the d-contraction (Gram) trick: the final output
            # only needs sum_d cur2, and
            #   out2[b, m] = sum_{f,g} W2[fg, m] * P12[b, f, g],
            #   P12[b, f, g] = sum_d x[b, f, d] * cur1[b, g, d].
            # P12 is built on the PE with a block-diagonal x operand (bd rows
            # on partitions, contraction over the 16 d's of each batch).
            p12 = outpool.tile([128, NCHUNK * NTILE * 9 * F0], DT, tag="p12")

            def do_l2p(c, cur1, xbdt):
                for t in range(NTILE):
                    pstc = pstbpool.tile([128, 128], DT, tag="pstb",
                                        name=f"pstc_{c}_{t}")
                    nc.tensor.transpose(
                        pstc[:], cur1[:, t * 128 : (t + 1) * 128], idb[:]
                    )
                    c1bd = curpool.tile([128, 128], DT, tag="c1bd",
                                        name=f"c1bd_{c}_{t}")
                    nc.scalar.copy(c1bd[:], pstc[:])
                    psa = psapool.tile([128, 9 * F0], DTF, tag="psa",
                                       name=f"psa_{c}_{t}")
                    nc.tensor.matmul(
                        psa[:], c1bd[:], xbdt[:, t, :],
                        start=True, stop=True,
                    )
                    off = (c * NTILE + t) * 9 * F0
                    nc.scalar.copy(p12[:, off : off + 9 * F0], psa[:])

            def flush_out(h):
                for k in range(3):
                    pst = pstpool.tile([128, 128], DTF, tag="pst",
                                       name=f"pst_{k}_{h}")
                    nc.tensor.transpose(
                        pst[:], outacc[k][:, h * 128 : (h + 1) * 128], ids[:]
                    )
                    ot = curpool.tile([128, 128], DTF, tag="otile",
                                      name=f"ot_{k}_{h}")
                    nc.scalar.copy(ot[:], pst[:])
                    nc.scalar.dma_start(
                        out=out[h * 128 : (h + 1) * 128, k * M : (k + 1) * M],
                        in_=ot[:],
                    )

            # stage B of the L2 Gram trick, over a half-range of chunks:
            # out2^T[m, b-half] = sum_f w2_f^T @ p12[:, (c, t, b, f)]
            pso2 = pso2pool.tile([128, BPC], DTF, tag="pso2")
            p12v = p12[:].rearrange("p (ct s f) -> p ct s f", s=9, f=F0)

            def stage_b(c0, c1):
                nt0, nt1 = c0 * NTILE, c1 * NTILE
                b0, b1 = c0 * BPCH, c1 * BPCH
                for f in range(F0):
                    nc.tensor.matmul(
                        pso2[:, b0:b1], w2s[:, f, :],
                        p12v[:, nt0:nt1, 0:8, f],
                        start=(f == 0), stop=(f == F0 - 1),
                    )
                nc.scalar.copy(outacc[2][:, b0:b1], pso2[:, b0:b1])

            # software pipeline, one-chunk skew on cur0:
            #   iter c: prefetch(c+2) | L1(c) groups 0..2 | L0(c+1) |
            #           L1(c) groups 3.. | reduce/copy tails | L2P(c)
            # so the cur0(c+1) bounce+replication DMA chain hides under the
            # back half of L1(c) and the L2 P-build.
            zin0_c, zin1_c, xrt_c, xbdt_c = pref
            zin0_n, zin1_n, xrt_n, xbdt_n = issue_chunk_dmas(1)
            ps0_c = do_l0A(0, zin0_c, zin1_c)
            rept_c = do_l0B(0, ps0_c)
            for c in range(NCHUNK):
                if c + 2 < NCHUNK:
                    pf2 = issue_chunk_dmas(c + 2)
                ps1 = pspool.tile([128, L], DTF, tag="ps", name=f"ps1_{c}")
                for fh in range(3):
                    l1_group(c, ps1, rept_c, xrt_c, fh)
                if c + 1 < NCHUNK:
                    ps0_n = do_l0A(c + 1, zin0_n, zin1_n)
                for fh in range(3, TF):
                    l1_group(c, ps1, rept_c, xrt_c, fh)
                cur1_c = do_l1B(c, ps1)
                if c + 1 < NCHUNK:
                    rept_n = do_l0B(c + 1, ps0_n)
                do_l2p(c, cur1_c, xbdt_c)
                if c == 3:
                    stage_b(0, 4)
                if c == 5:
                    stage_b(4, 6)
                if c + 1 < NCHUNK:
                    rept_c = rept_n
                    zin0_c, zin1_c, xrt_c, xbdt_c = zin0_n, zin1_n, xrt_n, xbdt_n
                if c + 2 < NCHUNK:
                    zin0_n, zin1_n, xrt_n, xbdt_n = pf2

            # stage B last quarter; earlier parts were emitted inside the loop.
            stage_b(6, 8)
            flush_out(0)
            flush_out(1)


    nc.compile()
    return nc


def _host_prep(inputs, f0, f1, f2):
    """Per-core input maps. Pure layout/cast/index-gather, no FLOP offload
    (except the W0 symmetrization, which is weight preprocessing)."""
    x = np.asarray(inputs)

    # symmetrized triangular W0: rows (f, g) f<=g
    f0n = np.asarray(f0).reshape(F0, F0, M)
    fi, gi = np.triu_indices(F0)
    w0t = f0n[fi, gi] + np.where((fi != gi)[:, None], f0n[gi, fi], 0.0)
    w0 = np.zeros((K0P, M), dtype=BF16)
    w0[:K0] = w0t.astype(BF16)

    # layer-1 weights permuted to the pair-split (t, p) row order
    w1 = np.asarray(f1).astype(np.float32)
    tt = np.repeat(np.arange(NKT1), 128)
    pp = np.tile(np.arange(128), NKT1)
    fidx1 = (tt // TG) * PF + pp // PG
    gidx1 = (tt % TG) * PG + pp % PG
    valid1 = fidx1 < F0
    rows = np.clip(fidx1, 0, F0 - 1) * M + gidx1
    w1perm = np.where(valid1[:, None], w1[rows], 0.0).astype(BF16)

    w2 = np.asarray(f2).astype(BF16)
    ident = np.eye(128, dtype=np.float32)
    identb = np.eye(128, dtype=BF16)

    # layer-0 z-factor gather indices (triangular, k-row = tile*128 + p)
    pidx = np.arange(K0P)
    fidx = np.zeros(K0P, np.int64)
    gidx = np.zeros(K0P, np.int64)
    fidx[:K0], gidx[:K0] = fi, gi
    valid = (pidx < K0).astype(BF16)[:, None]

    # layer-1 xrep row map: xrep[p, f_hi] = x row f_hi*PF + p//PG (<F0)
    prow = np.arange(128)
    fmap = np.arange(TF)[None, :] * PF + (prow // PG)[:, None]   # [128, TF]
    vmap = (fmap < F0)[..., None]

    maps = []
    for c in range(NCORES):
        xs = x[c * BPC : (c + 1) * BPC]                    # [256, 39, 16]
        xTf = np.ascontiguousarray(
            xs.transpose(1, 0, 2).reshape(F0, R)
        ).astype(BF16)                                     # [39, R]
        # layer-0 factors [K0P, R] -> chunk-major [NCHUNK, 128, NKT0, L]
        z0a = (xTf[gidx] * valid).reshape(NKT0, 128, NCHUNK, L)
        z0b = (xTf[fidx] * valid).reshape(NKT0, 128, NCHUNK, L)
        zin0c = np.ascontiguousarray(z0a.transpose(2, 1, 0, 3))
        zin1c = np.ascontiguousarray(z0b.transpose(2, 1, 0, 3))
        # layer-1 x-factor, 32x partition-replicated: [NCHUNK, 128, TF, L]
        xg = np.where(vmap, xTf[np.clip(fmap, 0, F0 - 1)], BF16(0))
        xrepc = np.ascontiguousarray(
            xg.reshape(128, TF, NCHUNK, L).transpose(2, 0, 1, 3)
        )
        xbd_full = xs.transpose(0, 2, 1).reshape(R, F0).astype(BF16)
        xbdt = xbd_full.reshape(NCHUNK, NTILE, 128, F0).transpose(0, 2, 1, 3)
        xbdh = np.zeros((NCHUNK, 128, NTILE, 9 * F0), dtype=BF16)
        p = np.arange(128)
        for s in range(8):
            rows_s = p[p // 16 == s]
            xbdh[:, rows_s, :, s * F0 : (s + 1) * F0] = xbdt[:, rows_s, :, :]
        maps.append(
            dict(zin0=zin0c, zin1=zin1c, w0=w0, w1p=w1perm, w2=w2,
                 ident=ident, identb=identb, xbd=xbdh, xrep=xrepc)
        )
    return maps


def kernel(**inputs) -> np.ndarray:
    if "nc" not in _CACHE:
        _CACHE["nc"] = _build_program()
    nc = _CACHE["nc"]
    maps = _host_prep(inputs["inputs"], inputs["f0"], inputs["f1"], inputs["f2"])
    res = run_bass_kernel_spmd(nc, maps, list(range(NCORES)))
    return np.concatenate([res.results[c]["out"] for c in range(NCORES)], axis=0)


if __name__ == "__main__":
    rng = np.random.default_rng(0)
    ins = {
        "inputs": rng.standard_normal((B, F0, D), dtype=np.float32),
        "f0": (rng.standard_normal((F0 * F0, M)) * 0.05).astype(np.float32),
        "f1": (rng.standard_normal((F0 * M, M)) * 0.05).astype(np.float32),
        "f2": (rng.standard_normal((F0 * M, M)) * 0.05).astype(np.float32),
    }
    out = kernel(**ins)
    print("out", out.shape, out.dtype)


# revision 9
# speedup vs baseline: 1.5713x; 1.3424x over previous
"""CIN (Compressed Interaction Network) kernel for Trainium2, 8-core data parallel.

Math (per batch row b, embedding dim d — R = B*D independent rows):
  layer k: cur_k[m, (b,d)] = sum_{f,g} W_k[f*G+g, m] * x0[f,(b,d)] * x_{k}[g,(b,d)]
  output  = concat_k( sum_d cur_k )    -> [B, 384]

Device strategy (per core, batch-sharded B/8 = 256 -> R = 4096 rows):
  * Everything lives feature-on-partitions: cur_k^T [128, R] etc.
  * Layer 0 uses the x (x) x symmetry: W0 is host-symmetrized to the upper
    triangle (k: 1521 -> 780, padded 896) and the two z factors are
    host-gathered index patterns of x^T (pure indexing, no arithmetic).
  * Layer 1 uses a PAIR-SPLIT of the (f, g) contraction index:
      pair (f, g) -> partition p = (f_lo in 4, g_lo in 32),
                     k-tile  t = (f_hi in 10, g_hi in 4)   (f padded 39->40)
    so the z build z[p, t, l] = xrep[p, f_hi, l] * cur0rep[p, g_hi, l]
    needs x replicated only 32x (host layout) and cur0 replicated only 4x
    (on-device DRAM bounce + 4 broadcast-read DMAs), instead of the naive
    x-broadcast to all 128 partitions (128x replication, 41 MB of DMA).
    W1 is host-permuted to match the (t, p) row order.
  * Layer 2's output only needs sum_d, so it uses the Gram trick: per-batch
    P12[b,f,g] = sum_d x[b,f,d] cur1[b,g,d] built on the PE with a
    block-diagonal x operand, then out2 = W2^T vec(P12) (stage B).
  * ALL z-multiplies run on the DVE: concurrent GpSimd tensor ops slow the
    DVE ~2.2x via SBUF port contention (measured), making any DVE/GpSimd
    split net-negative.
  * Queue discipline: ALL per-chunk dma_starts ride the sync (SP) sequencer
    (it has no compute duties, so its in-order waits block nothing); the
    Act sequencer only issues engine copies. GpSimd issues no DMAs (SWDGE
    prep costs ~1us of Pool engine time each).
  * Software pipeline: inputs prefetched 2 chunks ahead; chunk c+1's L0 is
    interleaved into the middle of chunk c's L1 matmul stream, and the cur0
    copy + bounce + replication reads are emitted immediately after L0's
    matmuls so the DMA chain hides under the remaining L1 groups.
"""

import sys
import types

sys.path.insert(0, "/opt/trn_rl_repo")

# The image's antenv package lacks axon_hooks; bass_utils imports it if
# BASS_TRACE is set in the environment. Seed a benign stub so that path
# degrades to "no tracing" instead of ModuleNotFoundError.
if "antenv.axon_hooks" not in sys.modules:
    _ah = types.ModuleType("antenv.axon_hooks")
    _ah.get_axon_ntff_profile_hook = lambda: None
    _ah.set_axon_ntff_profile_hook = lambda h: None
    sys.modules["antenv.axon_hooks"] = _ah

import numpy as np
import ml_dtypes

import concourse.bass as bass
import concourse.mybir as mybir
from concourse import bacc
from concourse.tile import TileContext
from concourse.bass_utils import run_bass_kernel_spmd

BF16 = ml_dtypes.bfloat16

B, F0, D = 2048, 39, 16
M = 128                      # layer width (all three layers)
NCORES = 8
BPC = B // NCORES            # batch per core = 256
R = BPC * D                  # rows per core = 4096
K0 = (F0 * (F0 + 1)) // 2    # 780 (triangular)
K0P = 896                    # padded to 7 k-tiles
NKT0 = K0P // 128            # 7
K0T = K0 - 6 * 128           # 12 valid rows in the 7th k-tile

# layer-1 pair split: p = (f_lo, g_lo), t = (f_hi, g_hi)
PF = 4                       # f_lo values per partition group
PG = 32                      # g_lo values per partition group
TF = 10                      # f_hi tiles (f padded 39 -> 40)
TG = 4                       # g_hi tiles (g = 128)
NKT1 = TF * TG               # 40 k-tiles

L = 512                      # bd-chunk (32 b x 16 d)
NCHUNK = R // L              # 8
BPCH = L // D                # 32 batches per chunk
NTILE = L // 128             # 4 bd-tiles of 128 rows per chunk

DT = mybir.dt.bfloat16
DTF = mybir.dt.float32

_CACHE = {}


def _build_program():
    nc = bacc.Bacc("TRN2", target_bir_lowering=False, debug=False,
                   num_devices=NCORES)

    # chunk-major layouts so per-partition DMA runs are contiguous
    zin0 = nc.declare_dram_parameter("zin0", [NCHUNK, 128, NKT0, L], DT,
                                     isOutput=False)
    zin1 = nc.declare_dram_parameter("zin1", [NCHUNK, 128, NKT0, L], DT,
                                     isOutput=False)
    xrep = nc.declare_dram_parameter("xrep", [NCHUNK, 128, TF, L], DT,
                                     isOutput=False)
    w0 = nc.declare_dram_parameter("w0", [K0P, M], DT, isOutput=False)
    w1p = nc.declare_dram_parameter("w1p", [NKT1 * 128, M], DT, isOutput=False)
    w2 = nc.declare_dram_parameter("w2", [F0 * M, M], DT, isOutput=False)
    ident = nc.declare_dram_parameter("ident", [128, 128], DTF, isOutput=False)
    identb = nc.declare_dram_parameter("identb", [128, 128], DT, isOutput=False)
    xbd = nc.declare_dram_parameter("xbd", [NCHUNK, 128, NTILE, 9 * F0], DT,
                                    isOutput=False)
    out = nc.declare_dram_parameter("out", [BPC, 3 * M], DTF, isOutput=True)

    # on-device bounce buffer for the cur0 4x partition replication
    c0scr = nc.dram_tensor("c0scr", (NCHUNK, 128, L), DT, kind="Internal")

    with TileContext(nc) as tc:
        with (
            tc.tile_pool(name="wpool", bufs=1) as wpool,
            tc.tile_pool(name="zin", bufs=3) as zinpool,
            tc.tile_pool(name="rep", bufs=2) as reppool,
            tc.tile_pool(name="zt", bufs=4) as zpool,
            tc.tile_pool(name="cur", bufs=3) as curpool,
            tc.tile_pool(name="outp", bufs=1) as outpool,
            tc.tile_pool(name="psum", bufs=3, space="PSUM") as pspool,
            tc.tile_pool(name="pst", bufs=1, space="PSUM") as pstpool,
            tc.tile_pool(name="pstb", bufs=2, space="PSUM") as pstbpool,
            tc.tile_pool(name="psa", bufs=1, space="PSUM") as psapool,
            tc.tile_pool(name="pso2", bufs=1, space="PSUM") as pso2pool,
        ):
            def issue_chunk_dmas(c, pieces=1):
                """Prefetch one chunk's L0 factors, L1 x-factor and L2
                block-diag x. No upstream waits -> safe anywhere on sync.
                The 7th zin k-tile only has 12 valid rows (780 = 6*128+12);
                load just those to trim ~0.5 MB/chunk."""
                zin0t = zinpool.tile([128, NKT0, L], DT, tag="zin0",
                                     name=f"zin0_{c}")
                zin1t = zinpool.tile([128, NKT0, L], DT, tag="zin1",
                                     name=f"zin1_{c}")
                if pieces > 1:
                    nc.sync.dma_start(out=zin0t[:, :3, :], in_=zin0[c, :, :3, :])
                    nc.sync.dma_start(out=zin1t[:, :3, :], in_=zin1[c, :, :3, :])
                    nc.sync.dma_start(out=zin0t[:, 3:6, :], in_=zin0[c, :, 3:6, :])
                    nc.sync.dma_start(out=zin1t[:, 3:6, :], in_=zin1[c, :, 3:6, :])
                else:
                    nc.sync.dma_start(out=zin0t[:, :6, :], in_=zin0[c, :, :6, :])
                    nc.sync.dma_start(out=zin1t[:, :6, :], in_=zin1[c, :, :6, :])
                nc.sync.dma_start(out=zin0t[0:K0T, 6:7, :],
                                  in_=zin0[c, 0:K0T, 6:7, :])
                nc.sync.dma_start(out=zin1t[0:K0T, 6:7, :],
                                  in_=zin1[c, 0:K0T, 6:7, :])
                xrt = zinpool.tile([128, TF, L], DT, tag="xrep",
                                   name=f"xrep_{c}")
                nc.sync.dma_start(out=xrt[:], in_=xrep[c])
                xbdt = zinpool.tile([128, NTILE, 9 * F0], DT, tag="xbd",
                                    name=f"xbd_{c}")
                nc.sync.dma_start(out=xbdt[:], in_=xbd[c])
                return zin0t, zin1t, xrt, xbdt

            G0 = [4, 2]          # layer-0 full k-tile TT groups (+ 12-row tail)

            # ---- small weights first (L0's matmuls need w0 immediately),
            # then the chunk-0 prefetch; w1p/w2 follow on the sync queue.
            w0s = wpool.tile([128, NKT0, M], DT, tag="w0")
            nc.sync.dma_start(out=w0s[:], in_=w0.rearrange("(t p) m -> p t m", p=128))
            ids = wpool.tile([128, 128], DTF, tag="ident")
            nc.scalar.dma_start(out=ids[:], in_=ident[:])
            idb = wpool.tile([128, 128], DT, tag="identb")
            nc.scalar.dma_start(out=idb[:], in_=identb[:])
            pref = issue_chunk_dmas(0, pieces=2)
            w1s = wpool.tile([128, NKT1, M], DT, tag="w1")
            nc.sync.dma_start(out=w1s[:], in_=w1p.rearrange("(t p) m -> p t m", p=128))
            w2s = wpool.tile([128, F0, M], DT, tag="w2")
            nc.scalar.dma_start(out=w2s[:], in_=w2.rearrange("(t p) m -> p t m", p=128))

            # per-layer output accumulators [128 m, BPC] fp32
            outacc = [
                outpool.tile([128, BPC], DTF, tag=f"oacc{k}", name=f"oacc{k}")
                for k in range(3)
            ]

            def do_l0A(c, zin0t, zin1t):
                """Layer 0 front half: z0 = zin0*zin1 on DVE, 7 PSUM-
                accumulated matmuls (the last over only 12 valid rows)."""
                ps0 = pspool.tile([128, L], DTF, tag="ps", name=f"ps0_{c}")
                kt = 0
                for gsz in G0:
                    z0t = zpool.tile([128, gsz, L], DT, tag="z",
                                     name=f"z0_{c}_{kt}")
                    nc.vector.tensor_mul(
                        z0t[:],
                        zin0t[:, kt : kt + gsz, :],
                        zin1t[:, kt : kt + gsz, :],
                    )
                    for j in range(gsz):
                        nc.tensor.matmul(
                            ps0[:],
                            w0s[:, kt + j, :],
                            z0t[:, j, :],
                            start=(kt + j == 0),
                            stop=False,
                        )
                    kt += gsz
                z0tt = zpool.tile([K0T, L], DT, tag="ztail",
                                  name=f"z0t_{c}")
                nc.vector.tensor_mul(
                    z0tt[:], zin0t[0:K0T, 6, :], zin1t[0:K0T, 6, :]
                )
                nc.tensor.matmul(
                    ps0[:], w0s[0:K0T, 6, :], z0tt[:],
                    start=False, stop=True,
                )
                return ps0

            def do_l0copy(c, ps0):
                """cur0 copy + the 4x partition replication via a DRAM
                bounce (all DMAs on sync). Emitted right after L0's matmuls
                so the chain overlaps the back half of the previous L1."""
                cur0 = curpool.tile([128, L], DT, tag="cur0", name=f"cur0_{c}")
                nc.scalar.copy(cur0[:], ps0[:])
                nc.sync.dma_start(out=c0scr[c], in_=cur0[:])
                rept = reppool.tile([128, TG, L], DT, tag="c0rep",
                                    name=f"c0rep_{c}")
                sview = c0scr[c].rearrange("(gh gl) l -> gl gh l", gl=PG)
                for f_lo in range(PF):
                    nc.sync.dma_start(
                        out=rept[f_lo * PG : (f_lo + 1) * PG, :, :], in_=sview
                    )
                return rept

            def do_l0red(c, ps0):
                nc.vector.tensor_reduce(
                    outacc[0][:, c * BPCH : (c + 1) * BPCH],
                    ps0[:].rearrange("p (b d) -> p b d", d=D),
                    axis=mybir.AxisListType.X,
                    op=mybir.AluOpType.add,
                )

            def l1_group(c, ps, rept, xrt, fh):
                """One f_hi group of layer 1: z build + TG matmuls."""
                zt = zpool.tile([128, TG, L], DT, tag="z", name=f"z1_{c}_{fh}")
                xb = xrt[:, fh, :].unsqueeze(1)
                nc.vector.tensor_mul(
                    zt[:], rept[:], xb.to_broadcast((128, TG, L)),
                )
                for gh in range(TG):
                    t = fh * TG + gh
                    nc.tensor.matmul(
                        ps[:],
                        w1s[:, t, :],
                        zt[:, gh, :],
                        start=(t == 0),
                        stop=(t == NKT1 - 1),
                    )

            def do_l1B(c, ps):
                """Layer 1 tail: out1 reduce + cur1 copy."""
                nc.vector.tensor_reduce(
                    outacc[1][:, c * BPCH : (c + 1) * BPCH],
                    ps[:].rearrange("p (b d) -> p b d", d=D),
                    axis=mybir.AxisListType.X,
                    op=mybir.AluOpType.add,
                )
                cur1 = curpool.tile([128, L], DT, tag="cur1", name=f"cur1_{c}")
                nc.scalar.copy(cur1[:], ps[:])
                return cur1

            # layer 2 via the d-contraction (Gram) trick: the final output
            # only needs sum_d cur2, and
            #   out2[b, m] = sum_{f,g} W2[fg, m] * P12[b, f, g],
            #   P12[b, f, g] = sum_d x[b, f, d] * cur1[b, g, d].
            # P12 is built on the PE with a block-diagonal x operand (bd rows
            # on partitions, contraction over the 16 d's of each batch).
            p12 = outpool.tile([128, NCHUNK * NTILE * 9 * F0], DT, tag="p12")

            def do_l2p(c, cur1, xbdt):
                for t in range(NTILE):
                    pstc = pstbpool.tile([128, 128], DT, tag="pstb",
                                        name=f"pstc_{c}_{t}")
                    nc.tensor.transpose(
                        pstc[:], cur1[:, t * 128 : (t + 1) * 128], idb[:]
                    )
                    c1bd = curpool.tile([128, 128], DT, tag="c1bd",
                                        name=f"c1bd_{c}_{t}")
                    nc.scalar.copy(c1bd[:], pstc[:])
                    psa = psapool.tile([128, 9 * F0], DTF, tag="psa",
                                       name=f"psa_{c}_{t}")
                    nc.tensor.matmul(
                        psa[:], c1bd[:], xbdt[:, t, :],
                        start=True, stop=True,
                    )
                    off = (c * NTILE + t) * 9 * F0
                    nc.scalar.copy(p12[:, off : off + 9 * F0], psa[:])

            def flush_out(h):
                for k in range(3):
                    pst = pstpool.tile([128, 128], DTF, tag="pst",
                                       name=f"pst_{k}_{h}")
                    nc.tensor.transpose(
                        pst[:], outacc[k][:, h * 128 : (h + 1) * 128], ids[:]
                    )
                    ot = curpool.tile([128, 128], DTF, tag="otile",
                                      name=f"ot_{k}_{h}")
                    nc.scalar.copy(ot[:], pst[:])
                    nc.scalar.dma_start(
                        out=out[h * 128 : (h + 1) * 128, k * M : (k + 1) * M],
                        in_=ot[:],
                    )

            # stage B of the L2 Gram trick, over a half-range of chunks:
            # out2^T[m, b-half] = sum_f w2_f^T @ p12[:, (c, t, b, f)]
            pso2 = pso2pool.tile([128, BPC], DTF, tag="pso2")
            p12v = p12[:].rearrange("p (ct s f) -> p ct s f", s=9, f=F0)

            def stage_b(c0, c1):
                nt0, nt1 = c0 * NTILE, c1 * NTILE
                b0, b1 = c0 * BPCH, c1 * BPCH
                for f in range(F0):
                    nc.tensor.matmul(
                        pso2[:, b0:b1], w2s[:, f, :],
                        p12v[:, nt0:nt1, 0:8, f],
                        start=(f == 0), stop=(f == F0 - 1),
                    )
                nc.scalar.copy(outacc[2][:, b0:b1], pso2[:, b0:b1])

            # software pipeline, one-chunk skew on cur0:
            #   iter c: prefetch(c+2) | L1(c) groups 0..2 | L0(c+1) matmuls |
            #           cur0(c+1) copy+bounce+replication | L1(c) groups 3.. |
            #           reduce tails | L2P(c)
            # so the cur0(c+1) bounce+replication DMA chain hides under the
            # back half of L1(c) and the L2 P-build.
            zin0_c, zin1_c, xrt_c, xbdt_c = pref
            zin0_n, zin1_n, xrt_n, xbdt_n = issue_chunk_dmas(1)
            ps0_c = do_l0A(0, zin0_c, zin1_c)
            rept_c = do_l0copy(0, ps0_c)
            do_l0red(0, ps0_c)
            for c in range(NCHUNK):
                if c + 2 < NCHUNK:
                    pf2 = issue_chunk_dmas(c + 2)
                ps1 = pspool.tile([128, L], DTF, tag="ps", name=f"ps1_{c}")
                for fh in range(3):
                    l1_group(c, ps1, rept_c, xrt_c, fh)
                if c + 1 < NCHUNK:
                    ps0_n = do_l0A(c + 1, zin0_n, zin1_n)
                    rept_n = do_l0copy(c + 1, ps0_n)
                for fh in range(3, TF):
                    l1_group(c, ps1, rept_c, xrt_c, fh)
                cur1_c = do_l1B(c, ps1)
                if c + 1 < NCHUNK:
                    do_l0red(c + 1, ps0_n)
                do_l2p(c, cur1_c, xbdt_c)
                if c == 3:
                    stage_b(0, 4)
                if c == 5:
                    stage_b(4, 6)
                if c + 1 < NCHUNK:
                    rept_c = rept_n
                    zin0_c, zin1_c, xrt_c, xbdt_c = zin0_n, zin1_n, xrt_n, xbdt_n
                if c + 2 < NCHUNK:
                    zin0_n, zin1_n, xrt_n, xbdt_n = pf2

            # stage B last quarter; earlier parts were emitted inside the loop.
            stage_b(6, 8)
            flush_out(0)
            flush_out(1)


    nc.compile()
    return nc


def _host_prep(inputs, f0, f1, f2):
    """Per-core input maps. Pure layout/cast/index-gather, no FLOP offload
    (except the W0 symmetrization, which is weight preprocessing)."""
    x = np.asarray(inputs)

    # symmetrized triangular W0: rows (f, g) f<=g
    f0n = np.asarray(f0).reshape(F0, F0, M)
    fi, gi = np.triu_indices(F0)
    w0t = f0n[fi, gi] + np.where((fi != gi)[:, None], f0n[gi, fi], 0.0)
    w0 = np.zeros((K0P, M), dtype=BF16)
    w0[:K0] = w0t.astype(BF16)

    # layer-1 weights permuted to the pair-split (t, p) row order
    w1 = np.asarray(f1).astype(np.float32)
    tt = np.repeat(np.arange(NKT1), 128)
    pp = np.tile(np.arange(128), NKT1)
    fidx1 = (tt // TG) * PF + pp // PG
    gidx1 = (tt % TG) * PG + pp % PG
    valid1 = fidx1 < F0
    rows = np.clip(fidx1, 0, F0 - 1) * M + gidx1
    w1perm = np.where(valid1[:, None], w1[rows], 0.0).astype(BF16)

    w2 = np.asarray(f2).astype(BF16)
    ident = np.eye(128, dtype=np.float32)
    identb = np.eye(128, dtype=BF16)

    # layer-0 z-factor gather indices (triangular, k-row = tile*128 + p)
    pidx = np.arange(K0P)
    fidx = np.zeros(K0P, np.int64)
    gidx = np.zeros(K0P, np.int64)
    fidx[:K0], gidx[:K0] = fi, gi
    valid = (pidx < K0).astype(BF16)[:, None]

    # layer-1 xrep row map: xrep[p, f_hi] = x row f_hi*PF + p//PG (<F0)
    prow = np.arange(128)
    fmap = np.arange(TF)[None, :] * PF + (prow // PG)[:, None]   # [128, TF]
    vmap = (fmap < F0)[..., None]

    maps = []
    for c in range(NCORES):
        xs = x[c * BPC : (c + 1) * BPC]                    # [256, 39, 16]
        xTf = np.ascontiguousarray(
            xs.transpose(1, 0, 2).reshape(F0, R)
        ).astype(BF16)                                     # [39, R]
        # layer-0 factors [K0P, R] -> chunk-major [NCHUNK, 128, NKT0, L]
        z0a = (xTf[gidx] * valid).reshape(NKT0, 128, NCHUNK, L)
        z0b = (xTf[fidx] * valid).reshape(NKT0, 128, NCHUNK, L)
        zin0c = np.ascontiguousarray(z0a.transpose(2, 1, 0, 3))
        zin1c = np.ascontiguousarray(z0b.transpose(2, 1, 0, 3))
        # layer-1 x-factor, 32x partition-replicated: [NCHUNK, 128, TF, L]
        xg = np.where(vmap, xTf[np.clip(fmap, 0, F0 - 1)], BF16(0))
        xrepc = np.ascontiguousarray(
            xg.reshape(128, TF, NCHUNK, L).transpose(2, 0, 1, 3)
        )
        xbd_full = xs.transpose(0, 2, 1).reshape(R, F0).astype(BF16)
        xbdt = xbd_full.reshape(NCHUNK, NTILE, 128, F0).transpose(0, 2, 1, 3)
        xbdh = np.zeros((NCHUNK, 128, NTILE, 9 * F0), dtype=BF16)
        p = np.arange(128)
        for s in range(8):
            rows_s = p[p // 16 == s]
            xbdh[:, rows_s, :, s * F0 : (s + 1) * F0] = xbdt[:, rows_s, :, :]
        maps.append(
            dict(zin0=zin0c, zin1=zin1c, w0=w0, w1p=w1perm, w2=w2,
                 ident=ident, identb=identb, xbd=xbdh, xrep=xrepc)
        )
    return maps


def kernel(**inputs) -> np.ndarray:
    if "nc" not in _CACHE:
        _CACHE["nc"] = _build_program()
    nc = _CACHE["nc"]
    maps = _host_prep(inputs["inputs"], inputs["f0"], inputs["f1"], inputs["f2"])
    res = run_bass_kernel_spmd(nc, maps, list(range(NCORES)))
    return np.concatenate([res.results[c]["out"] for c in range(NCORES)], axis=0)


if __name__ == "__main__":
    rng = np.random.default_rng(0)
    ins = {
        "inputs": rng.standard_normal((B, F0, D), dtype=np.float32),
        "f0": (rng.standard_normal((F0 * F0, M)) * 0.05).astype(np.float32),
        "f1": (rng.standard_normal((F0 * M, M)) * 0.05).astype(np.float32),
        "f2": (rng.standard_normal((F0 * M, M)) * 0.05).astype(np.float32),
    }
    out = kernel(**ins)
    print("out", out.shape, out.dtype)
